# revision 2
# baseline (speedup 1.0000x reference)
"""Trainium2 Bass kernel for nn_CABlock_26912265077025.

Architecture: CA-gating block (pools -> conv -> sigmoid gates -> x*gd*gh*gw)
followed by a 12000->4096->512->3 MLP and row L2-normalization.

Strategy: data parallel over batch (512 rows/core). mm1 (the 12032x4096
GEMM, ~95% of FLOPs) runs as THREE fp8e4 DoubleRow passes sharing one PSUM
accumulation at product scale 2^13:
    acc = z8@w8 + z8@dw8 + dz8@w8          (dz/dw = fp8 residuals)
which recovers ~bf16 accuracy (measured 7.7e-3 final rel RMS) at the fp8
DoubleRow rate. z8/dz8 are produced on-chip per k-pair: ACT exp -> DVE mul
(bf16) -> Pool cast (fp8) -> DVE sub (fp8), while the PE runs M_HEAD m-tiles
of mm1 synchronized behind the production frontier. Pools use fp8 DoubleRow
on a separately-loaded fp8 copy of x so gating starts after only ~7MB of
DMA. Gates use one 128-row channel-grouped log-sigmoid selection matmul per
k-tile. mm2/mm3 run in bf16. DMA is spread over 4 issue queues (SP, ACT,
DVE, Pool) because a queue's sequencer is held for each transfer's duration.
"""

from contextlib import ExitStack

import numpy as np
import ml_dtypes

import concourse.bass as bass
import concourse.mybir as mybir
import concourse.tile as tile
from concourse import bacc
from concourse.bass_utils import run_bass_kernel_spmd

N_CORES = 8
B_TOT = 4096
BS = B_TOT // N_CORES          # 512 batch rows per core
F = 12000
NK = 94                        # k-tiles of 128 features
NP = 47                        # k-pairs (DoubleRow processes 2 k-tiles)
FP = NK * 128                  # 12032
H1, H2 = 4096, 512
NM1 = H1 // 128                # 32
NM2 = H2 // 128                # 4
NK2 = H1 // 128                # 32 mm2 k-tiles
M_HEAD = 5                     # m-tiles computed behind the phase-C frontier
CH = 8                         # w1 chunk size in k-pairs
NCH = (NP + CH - 1) // CH      # 6 chunks (last covers 7 pairs)
XG = 4                         # x-group size in k-pairs
NXG = (NP + XG - 1) // XG      # 12 groups (last has 3)
SELG = 8                       # ssel group size in k-tiles
NSELG = (NK + SELG - 1) // SELG  # 12 groups (last has 6)
LN8 = float(np.log(8.0))

f32 = mybir.dt.float32
bf16 = mybir.dt.bfloat16
f8 = mybir.dt.float8e4
AF = mybir.ActivationFunctionType
DR = mybir.MatmulPerfMode.DoubleRow

E4 = ml_dtypes.float8_e4m3
BF = ml_dtypes.bfloat16

_NC_CACHE = {}


def build_nc(debug=False):
    nc = bacc.Bacc(None, target_bir_lowering=False)

    x8_d = nc.dram_tensor("x8", [NXG, 128, XG, 2, BS], f8, kind="ExternalInput")
    xbf_d = nc.dram_tensor("xbf", [NXG, 128, XG, 2, BS], bf16, kind="ExternalInput")
    wp_d = nc.dram_tensor("wp8", [128, NP, 2, 64], f8, kind="ExternalInput")
    rm_d = nc.dram_tensor("rmab", [50, 2, 128], bf16, kind="ExternalInput")
    ssel_d = nc.dram_tensor("ssel", [NSELG, 128, SELG, 128], bf16, kind="ExternalInput")
    w1_d = nc.dram_tensor("w1q", [NM1, 128, NCH * CH, 2, 2, 128], f8, kind="ExternalInput")
    w2_d = nc.dram_tensor("w2k", [NK2, 128, NM2, 128], bf16, kind="ExternalInput")
    w3_d = nc.dram_tensor("w3h", [128, NM2, 3], bf16, kind="ExternalInput")
    b1_d = nc.dram_tensor("b1g", [128, NM1], f32, kind="ExternalInput")
    b2_d = nc.dram_tensor("b2g", [128, NM2], f32, kind="ExternalInput")
    b3_d = nc.dram_tensor("b3g", [3, 1], f32, kind="ExternalInput")
    out_d = nc.dram_tensor("out", [3, BS], f32, kind="ExternalOutput")
    if debug:
        dbg_y = nc.dram_tensor("dbg_y", [50, BS], bf16, kind="ExternalOutput")
        dbg_lpa = nc.dram_tensor("dbg_lpa", [128, BS], bf16, kind="ExternalOutput")
        dbg_lpb = nc.dram_tensor("dbg_lpb", [128, BS], bf16, kind="ExternalOutput")
        dbg_gt = nc.dram_tensor("dbg_gt", [128, 2, 2, BS], bf16, kind="ExternalOutput")
        dbg_z8 = nc.dram_tensor("dbg_z8", [128, 2, 2, BS], f8, kind="ExternalOutput")
        dbg_zr8 = nc.dram_tensor("dbg_zr8", [128, 2, 2, BS], f8, kind="ExternalOutput")
        dbg_z1 = nc.dram_tensor("dbg_z1", [128, BS], bf16, kind="ExternalOutput")

    with tile.TileContext(nc) as tc, ExitStack() as ctx:
        consts = ctx.enter_context(tc.tile_pool(name="consts", bufs=1))

        b1_sb = consts.tile([128, NM1], f32)
        b2_sb = consts.tile([128, NM2], f32)
        b3_sb = consts.tile([3, 1], f32)
        w3_sb = consts.tile([128, NM2, 3], bf16)
        rm_sb = consts.tile([50, 2, 128], bf16)
        ln8_sb = consts.tile([128, 1], f32)
        ones31 = consts.tile([3, 1], f32)
        ones13 = consts.tile([1, 3], f32)
        nc.any.memset(ln8_sb[:], LN8)
        nc.any.memset(ones31[:], 1.0)
        nc.any.memset(ones13[:], 1.0)

        # persistent pools (lifetime: pre-A through phase E, LIFO-outermost)
        NB = (NP + 1) // 2  # 24 two-pair production blocks (last half-padded)
        zstack = ExitStack()
        z8pool = zstack.enter_context(tc.tile_pool(name="z8p", bufs=NB))
        zr8pool = zstack.enter_context(tc.tile_pool(name="zr8p", bufs=NB))
        z8s, zr8s, z1s = [], [], []

        # phase C..D pools
        dstack = ExitStack()
        psm = dstack.enter_context(tc.tile_pool(name="psm", bufs=1, space="PSUM"))

        # phase C-only pools (close at end of C)
        cstack = ExitStack()
        lpp = cstack.enter_context(tc.tile_pool(name="lpp", bufs=1))
        sselp = cstack.enter_context(tc.tile_pool(name="sselp", bufs=2))
        xbfp = cstack.enter_context(tc.tile_pool(name="xbfp", bufs=2))
        gtp = cstack.enter_context(tc.tile_pool(name="gtp", bufs=2))
        zfp = cstack.enter_context(tc.tile_pool(name="zfp", bufs=3))
        psg = cstack.enter_context(tc.tile_pool(name="psg", bufs=3, space="PSUM"))
        w1hp = cstack.enter_context(tc.tile_pool(name="w1hp", bufs=2))
        psmh = cstack.enter_context(tc.tile_pool(name="psmh", bufs=1, space="PSUM"))

        sseltiles = []
        xbftiles = []

        def xbf_dma(g, qeng):
            if g >= NXG:
                return
            xt = xbfp.tile([128, XG, 2, BS], bf16, tag="xbf")
            qeng.dma_start(xt[:], xbf_d[g])
            xbftiles.append(xt)

        def ssel_dma(g, qeng):
            if g >= NSELG:
                return
            st = sselp.tile([128, SELG, 128], bf16, tag="ssel")
            qeng.dma_start(st[:], ssel_d[g])
            sseltiles.append(st)

        # first ssel group rides the ACT queue (after the odd x8 groups);
        # xbf groups all ride SP right behind the even x8 groups
        pass

        # ---------------- Phase A+B: pools (fp8 DoubleRow) -> Lp tiles
        LpA = lpp.tile([128, BS], bf16, tag="LpA")
        LpB = lpp.tile([128, BS], bf16, tag="LpB")
        # phase A/B PSUM rides the phase-C pools: ypre rotates through the
        # gate-psum tag, Ta/Tb through the head-accumulator tags (same shape)
        with (
            tc.tile_pool(name="x8p", bufs=2) as x8pool,
            tc.tile_pool(name="wpp", bufs=1) as wpp,
        ):
            wpt = wpp.tile([128, NP, 2, 64], f8)
            nc.gpsimd.dma_start(wpt[:], wp_d[:])
            ypre = psg.tile([64, BS], f32, tag="gp", name="ypre")
            for g in range(NXG):
                cnt = min(XG, NP - g * XG)
                x8t = x8pool.tile([128, XG, 2, BS], f8, tag="x8")
                # split the critical x8 stream across SP and ACT queues so
                # issue overhead pipelines with transfers
                (nc.sync if g % 2 == 0 else nc.scalar).dma_start(x8t[:], x8_d[g])
                for i in range(cnt):
                    p = g * XG + i
                    nc.tensor.matmul(
                        ypre[:], wpt[:, p], x8t[:, i],
                        start=(p == 0), stop=(p == NP - 1),
                        perf_mode=DR,
                    )
            ssel_dma(0, nc.scalar)
            nc.scalar.dma_start(rm_sb[:], rm_d[:])
            # y = relu(ypre * 2^-8) in bf16; T = rm^T @ y; Lp = softplus(-T)
            # (y and ea/eb borrow slots from the phase-C rolling pools)
            y_t = zfp.tile([128, 2, BS], bf16, tag="zf", name="y_t")
            y_sb = y_t[0:50, 0]
            nc.scalar.activation(y_sb, ypre[0:50], AF.Relu, scale=float(2.0 ** -8))
            Ta = psmh.tile([128, BS], f32, tag="mm1h_0", bufs=1, name="Ta")
            Tb = psmh.tile([128, BS], f32, tag="mm1h_1", bufs=1, name="Tb")
            nc.tensor.matmul(Ta[:], rm_sb[:, 0], y_sb)
            nc.tensor.matmul(Tb[:], rm_sb[:, 1], y_sb)
            if debug:
                nc.gpsimd.dma_start(dbg_y[:], y_sb)
            e_t = gtp.tile([128, 2, BS], bf16, tag="gt", name="e_t")
            nc.scalar.activation(e_t[:, 0], Ta[:], AF.Exp, scale=-1.0)
            nc.scalar.activation(e_t[:, 1], Tb[:], AF.Exp, scale=-1.0)
            nc.scalar.activation(LpA[:], e_t[:, 0], AF.Ln, bias=1.0)
            nc.scalar.activation(LpB[:], e_t[:, 1], AF.Ln, bias=1.0)

        # ---------------- Phase C: gating + z8/dz8 production + mm1 head
        def w1chunk(m, c0, cnt, pool, tag, qeng, bufs):
            wt = pool.tile([128, CH, 2, 2, 128], f8, tag=tag, bufs=bufs)
            nm = min(cnt, NP - c0)
            qeng.dma_start(wt[:, :nm], w1_d[m, :, c0 : c0 + nm])
            return wt

        def mm1(acc, wt, j, t, zt, start, stop):
            # t: 0 = z8@w8, 1 = z8@dw8, 2 = dz8@w8 (stationary idx 1 only for t=1)
            nc.tensor.matmul(
                acc[:], wt[:, j, 1 if t == 1 else 0], zt[:],
                start=start, stop=stop,
                perf_mode=DR, skip_group_check=True,
            )

        # heads 0..2 use the psmh banks; heads 3..4 borrow the two mm1d
        # banks, which phase D only needs after the head accs are evicted
        accs_head = [
            psmh.tile([128, BS], f32, tag=f"mm1h_{m}", bufs=1, name=f"acch_{m}")
            for m in range(3)
        ] + [
            psm.tile([128, BS], f32, tag="mm1d", bufs=2, name=f"acch_{m}")
            for m in range(3, M_HEAD)
        ]
        # chunk 0 for the head m-tiles rides the (otherwise idle) Pool queue
        wts_head = {
            (m, 0): w1chunk(m, 0, CH, w1hp, f"w1h_{m}", nc.gpsimd, 2)
            for m in range(M_HEAD)
        }
        nc.gpsimd.dma_start(b1_sb[:], b1_d[:])
        nc.gpsimd.dma_start(b2_sb[:], b2_d[:])
        nc.gpsimd.dma_start(b3_sb[:], b3_d[:])
        nc.gpsimd.dma_start(w3_sb[:], w3_d[:])

        def evict_z1(m, acc):
            z1t = z8pool.tile([128, BS], bf16, tag="z1", bufs=6, name=f"z1_{m}")
            nc.scalar.activation(
                z1t[:], acc[:], AF.Relu,
                bias=b1_sb[:, m : m + 1], scale=float(2.0 ** -13),
            )
            z1s.append(z1t)
            if debug and m == 0:
                nc.gpsimd.dma_start(dbg_z1[:], z1t[:])

        def emit_pass1(q):
            for m in range(M_HEAD):
                mm1(accs_head[m], wts_head[(m, q // CH)], q % CH, 0, z8s[q],
                    q == 0, False)

        def emit_passes23(p, last=False):
            c, j = p // CH, p % CH
            for m in range(M_HEAD):
                wt = wts_head[(m, c)]
                mm1(accs_head[m], wt, j, 1, z8s[p], False, False)    # z8 @ dw8
                mm1(accs_head[m], wt, j, 2, zr8s[p], False,
                    last and m == M_HEAD - 1)                        # dz8 @ w8

        if debug:
            nc.gpsimd.dma_start(dbg_lpa[:], LpA[:])
            nc.gpsimd.dma_start(dbg_lpb[:], LpB[:])
        zcs = []
        for b in range(NB):
            # weave streamed inputs into SP with ~2-group lead time
            if b == 0:
                xbf_dma(0, nc.sync)
                xbf_dma(1, nc.sync)
            if b % 2 == 0:
                xbf_dma(b // 2 + 2, nc.sync)
            if b % 4 == 0:
                ssel_dma(b // 4 * 2 + 1, nc.sync)
                ssel_dma(b // 4 * 2 + 2, nc.sync)
                if (b // 4 + 1) * CH < NP:
                    for m in range(M_HEAD):
                        wts_head[(m, b // 4 + 1)] = w1chunk(
                            m, (b // 4 + 1) * CH, CH, w1hp, f"w1h_{m}", nc.sync, 2
                        )

            gt = gtp.tile([128, 2, 2, BS], bf16, tag="gt")
            for i in range(2):
                p = 2 * b + i
                if p >= NP:
                    nc.vector.memzero(gt[:, i])
                    continue
                for s in range(2):
                    k = 2 * p + s
                    gp = psg.tile([128, BS], f32, tag="gp")
                    Lp = LpA if k <= 61 else LpB
                    nc.tensor.matmul(
                        gp[:], sseltiles[k // SELG][:, k % SELG], Lp[:],
                        skip_group_check=True,
                    )
                    nc.scalar.activation(
                        gt[:, i, s], gp[:], AF.Exp, bias=ln8_sb[:, 0:1], scale=-1.0
                    )
            zf = zfp.tile([128, 2, 2, BS], bf16, tag="zf")
            g, h = b // 2, (b % 2) * 2
            nc.vector.tensor_mul(zf[:], xbftiles[g][:, h : h + 2], gt[:])
            z8b = z8pool.tile([128, 2, 2, BS], f8, tag="z8")
            nc.gpsimd.tensor_copy(z8b[:], zf[:])
            zr8b = zr8pool.tile([128, 2, 2, BS], f8, tag="zr8")
            # the sub for block b-1 is emitted AFTER mul(b): DVE is in-order,
            # so this keeps mul(b) from queueing behind a sub that waits on
            # the Pool cast (the cast overlaps the next mul instead)
            zcs.append((zf, z8b, zr8b))
            if b > 0:
                pzf, pz8, pzr = zcs[b - 1]
                nc.vector.tensor_sub(pzr[:], pzf[:], pz8[:])
            if debug and b == 0:
                nc.gpsimd.dma_start(dbg_gt[:], gt[:])
                nc.gpsimd.dma_start(dbg_z8[:], z8b[:])
            if debug and b == 1:
                nc.gpsimd.dma_start(dbg_zr8[:], zcs[0][2][:])
            for i in range(2):
                if 2 * b + i < NP:
                    z8s.append(z8b[:, i])
                    zr8s.append(zr8b[:, i])

            # mm1 head lags the production frontier by 6 pairs (pass 1) and
            # 8 pairs (passes 2/3): the production chain latency is ~2 pairs,
            # so the in-order PE stream never waits on a just-produced tile
            for i in range(2):
                q = 2 * b + i - 6
                if 0 <= q < NP:
                    emit_pass1(q)
                q = 2 * b + i - 8
                if 0 <= q < NP:
                    emit_passes23(q)
        pzf, pz8, pzr = zcs[NB - 1]
        nc.vector.tensor_sub(pzr[:], pzf[:], pz8[:])
        for q in range(NP - 5, NP):
            emit_pass1(q)
        for q in range(NP - 7, NP):
            emit_passes23(q, last=(q == NP - 1))
        for m in range(M_HEAD):
            evict_z1(m, accs_head[m])
        cstack.close()

        # ---------------- Phase D: remaining mm1 m-tiles (full speed).
        # Chunk DMAs are emitted two chunks ahead in the flat (m, c) sequence
        # so the SP queue fires each load exactly when its slot frees.
        estack = ExitStack()
        w1p = estack.enter_context(tc.tile_pool(name="w1p", bufs=3))
        w2p = estack.enter_context(tc.tile_pool(name="w2p", bufs=16))
        psm2 = estack.enter_context(tc.tile_pool(name="psm2", bufs=1, space="PSUM"))
        w2tiles = []
        acc2s = [
            psm2.tile([128, BS], f32, tag=f"mm2_{m2}", name=f"acc2_{m2}")
            for m2 in range(NM2)
        ]

        def emit_mm2(k2):
            for m2 in range(NM2):
                nc.tensor.matmul(
                    acc2s[m2][:], w2tiles[k2][:, m2], z1s[k2][:],
                    start=(k2 == 0), stop=(k2 == NK2 - 1),
                    skip_group_check=True,
                )
        chunks = [(m, c) for m in range(M_HEAD, NM1) for c in range(NCH)]
        wts_d = {}
        wts_d[chunks[0]] = w1chunk(chunks[0][0], chunks[0][1] * CH, CH, w1p, "w1d", nc.sync, 3)
        wts_d[chunks[1]] = w1chunk(chunks[1][0], chunks[1][1] * CH, CH, w1p, "w1d", nc.sync, 3)
        accs_d = {}
        for idx, (m, c) in enumerate(chunks):
            if idx + 2 < len(chunks):
                nm, ncc = chunks[idx + 2]
                wts_d[(nm, ncc)] = w1chunk(nm, ncc * CH, CH, w1p, "w1d", nc.sync, 3)
            if c == 0:
                accs_d[m] = psm.tile([128, BS], f32, tag="mm1d", bufs=2, name=f"accd_{m}")
                # w2 prefetch rides the (idle) Pool queue, 4 tiles per m-sweep
                for k2 in range(len(w2tiles), min(NK2, m + 2)):
                    w2t = w2p.tile([128, NM2, 128], bf16, tag="w2")
                    nc.gpsimd.dma_start(w2t[:], w2_d[k2])
                    w2tiles.append(w2t)
            acc, wt = accs_d[m], wts_d[(m, c)]
            cnt = min(CH, NP - c * CH)
            for j in range(cnt):
                p = c * CH + j
                mm1(acc, wt, j, 0, z8s[p], p == 0, False)
                mm1(acc, wt, j, 2, zr8s[p], False, False)
                mm1(acc, wt, j, 1, z8s[p], False, p == NP - 1)
            if c == 2:
                # interleave mm2 for already-evicted z1 tiles (k2 <= m-1)
                if m == M_HEAD:
                    for k2 in range(M_HEAD):
                        emit_mm2(k2)
                else:
                    emit_mm2(m - 1)
            if c == NCH - 1:
                evict_z1(m, accs_d[m])

        # ---------------- Phase E: mm2 (bf16) + Phase F: mm3 + normalize
        emit_mm2(NK2 - 1)
        with (
            tc.tile_pool(name="z2p", bufs=1) as z2p,
            tc.tile_pool(name="tailp", bufs=1) as tailp,
            tc.tile_pool(name="psf", bufs=1, space="PSUM") as psf,
        ):
            z2_tiles = []
            for m2 in range(NM2):
                z2t = z2p.tile([128, BS], bf16, tag=f"z2_{m2}")
                nc.scalar.activation(
                    z2t[:], acc2s[m2][:], AF.Relu, bias=b2_sb[:, m2 : m2 + 1]
                )
                z2_tiles.append(z2t)

            acc3 = psf.tile([3, BS], f32, tag="f")  # sps/inv3 reuse this bank
            for k3 in range(NM2):
                nc.tensor.matmul(
                    acc3[:], w3_sb[:, k3], z2_tiles[k3][:],
                    start=(k3 == 0), stop=(k3 == NM2 - 1),
                )
            z3 = tailp.tile([3, BS], f32)
            nc.vector.tensor_scalar_add(z3[:], acc3[:], b3_sb[:])
            sq = tailp.tile([3, BS], f32)
            nc.scalar.activation(sq[:], z3[:], AF.Square)
            sps = psf.tile([1, BS], f32, tag="f")
            nc.tensor.matmul(sps[:], ones31[:], sq[:])
            # 1/max(sqrt(s), 1e-12) = min(exp(-0.5*ln(s)), 1e12)
            lns = tailp.tile([1, BS], f32)
            nc.scalar.activation(lns[:], sps[:], AF.Ln)
            inv = tailp.tile([1, BS], f32)
            nc.scalar.activation(inv[:], lns[:], AF.Exp, scale=-0.5)
            nc.vector.tensor_scalar_min(inv[:], inv[:], 1e12)
            inv3 = psf.tile([3, BS], f32, tag="f")
            nc.tensor.matmul(inv3[:], ones13[:], inv[:])
            outt = tailp.tile([3, BS], f32)
            nc.vector.tensor_mul(outt[:], z3[:], inv3[:])
            nc.gpsimd.dma_start(out_d[:], outt[:])

        estack.close()
        dstack.close()
        zstack.close()

    nc.compile()
    return nc


def _prep_shared(conv_w, F_w, w1, b1, w2, b2, w3, b3):
    """Host-side weight layouts shared by all cores."""
    fa = np.arange(F)
    c_i = fa // 4000
    d_i = (fa // 400) % 10
    h_i = (fa // 40) % 10
    w_i = fa % 40

    # pooled conv (x8 fp8, weights scaled 2^8): ypre[j] = sum_f wp[f,j]*x8T[f,b]
    wp = np.zeros((FP, 64), np.float32)
    wp[fa, h_i] += conv_w[c_i] / 400.0 * 256.0
    wp[fa, 10 + w_i] += conv_w[c_i] / 100.0 * 256.0
    wp8 = np.ascontiguousarray(
        wp.reshape(NP, 2, 128, 64).transpose(2, 0, 1, 3)
    ).astype(E4)

    # rm150 channel-grouped: T[c*50+i] = F_w[c]*y[i](sh), T[c*50+10+w'] = F_w[c]*y[10+w']
    rm150 = np.zeros((50, 150), np.float32)
    for c in range(3):
        rm150[np.arange(10), c * 50 + np.arange(10)] = F_w[c]
        rm150[10 + np.arange(40), c * 50 + 10 + np.arange(40)] = F_w[c]
    rmab = np.stack([rm150[:, 0:128], rm150[:, 22:150]], axis=1)  # [50, 2, 128]
    rmab = np.ascontiguousarray(rmab).astype(BF)

    # per-k selection over the channel-grouped Lp window (A: rows 0..127 for
    # k<=61, B: rows 22..149 for k>=62); += handles the d==h coincidence
    sel = np.zeros((NK, 128, 128), np.float32)
    rows = np.stack([c_i * 50 + d_i, c_i * 50 + h_i, c_i * 50 + 10 + w_i])
    for k in range(NK):
        base = 0 if k <= 61 else 22
        jj = np.arange(k * 128, min((k + 1) * 128, F))
        for r3 in rows:
            np.add.at(sel[k], (r3[jj] - base, jj - k * 128), 1.0)
    selg = np.zeros((NSELG * SELG, 128, 128), np.float32)
    selg[:NK] = sel
    ssel = np.ascontiguousarray(
        selg.reshape(NSELG, SELG, 128, 128).transpose(0, 2, 1, 3)
    ).astype(BF)

    # w1: fp8 main + fp8 residual at shared scale 2^10, k-pair interleaved
    w1s = (w1.astype(np.float64) * 1024.0).astype(np.float32)
    w1p_ = np.zeros((H1, NCH * CH * 256), np.float32)
    w1p_[:, :F] = w1s[:, :F]
    w8 = w1p_.astype(E4)
    wr8 = (w1p_ - w8.astype(np.float32)).astype(E4)

    # [H1, FPc] -> [m, mcol, pair, s, p] -> [m, p, pair, s, mcol]
    def lay(a):
        return a.reshape(NM1, 128, NCH * CH, 2, 128).transpose(0, 4, 2, 3, 1)

    w1q = np.ascontiguousarray(np.stack([lay(w8), lay(wr8)], axis=3))
    # shape [NM1, 128, NCH*CH(pair), 2(t), 2(s), 128]

    w2k = np.ascontiguousarray(
        w2.reshape(NM2, 128, NK2, 128).transpose(2, 3, 0, 1)
    ).astype(BF)
    w3h = np.ascontiguousarray(w3.reshape(3, NM2, 128).transpose(2, 1, 0)).astype(BF)

    return {
        "wp8": wp8,
        "rmab": rmab,
        "ssel": ssel,
        "w1q": w1q,
        "w2k": w2k,
        "w3h": w3h,
        "b1g": np.ascontiguousarray(b1.reshape(NM1, 128).T),
        "b2g": np.ascontiguousarray(b2.reshape(NM2, 128).T),
        "b3g": np.ascontiguousarray(b3.reshape(3, 1)),
    }


def make_in_maps(x, conv_w, F_w, w1, b1, w2, b2, w3, b3):
    x = np.asarray(x, np.float32).reshape(B_TOT, F)
    shared = _prep_shared(
        np.asarray(conv_w, np.float32).reshape(3),
        np.asarray(F_w, np.float32).reshape(3),
        np.asarray(w1, np.float32),
        np.asarray(b1, np.float32),
        np.asarray(w2, np.float32),
        np.asarray(b2, np.float32),
        np.asarray(w3, np.float32),
        np.asarray(b3, np.float32),
    )
    in_maps = []
    for c in range(N_CORES):
        xs = x[c * BS : (c + 1) * BS]
        xt = np.zeros((NXG * XG * 2 * 128, BS), np.float32)
        xt[:F] = xs.T
        # [f, b] -> [g, k(XG), s, p, b] -> [g, p, k, s, b]
        xt = xt.reshape(NXG, XG, 2, 128, BS).transpose(0, 3, 1, 2, 4)
        m = dict(shared)
        m["x8"] = np.ascontiguousarray(xt).astype(E4)
        m["xbf"] = np.ascontiguousarray(xt).astype(BF)
        in_maps.append(m)
    return in_maps


def get_nc():
    if "nc" not in _NC_CACHE:
        _NC_CACHE["nc"] = build_nc()
    return _NC_CACHE["nc"]


def kernel(**inputs) -> np.ndarray:
    nc = get_nc()
    in_maps = make_in_maps(**inputs)
    res = run_bass_kernel_spmd(nc, in_maps, core_ids=list(range(N_CORES)))
    out = np.concatenate([r["out"] for r in res.results], axis=1)  # [3, 4096]
    return np.ascontiguousarray(out.T, dtype=np.float32)


# revision 3
# speedup vs baseline: 1.0084x; 1.0084x over previous
"""Trainium2 Bass kernel for nn_CABlock_26912265077025.

Architecture: CA-gating block (pools -> conv -> sigmoid gates -> x*gd*gh*gw)
followed by a 12000->4096->512->3 MLP and row L2-normalization.

Strategy: data parallel over batch (512 rows/core). mm1 (the 12032x4096
GEMM, ~95% of FLOPs) runs as THREE fp8e4 DoubleRow passes sharing one PSUM
accumulation at product scale 2^13:
    acc = z8@w8 + z8@dw8 + dz8@w8          (dz/dw = fp8 residuals)
which recovers ~bf16 accuracy (measured 7.7e-3 final rel RMS) at the fp8
DoubleRow rate. z8/dz8 are produced on-chip per k-pair: ACT exp -> DVE mul
(bf16) -> Pool cast (fp8) -> DVE sub (fp8), while the PE runs M_HEAD m-tiles
of mm1 synchronized behind the production frontier. Pools use fp8 DoubleRow
on a separately-loaded fp8 copy of x so gating starts after only ~7MB of
DMA. Gates use one 128-row channel-grouped log-sigmoid selection matmul per
k-tile. mm2/mm3 run in bf16. DMA is spread over 4 issue queues (SP, ACT,
DVE, Pool) because a queue's sequencer is held for each transfer's duration.
"""

from contextlib import ExitStack

import numpy as np
import ml_dtypes

import concourse.bass as bass
import concourse.mybir as mybir
import concourse.tile as tile
from concourse import bacc
from concourse.bass_utils import run_bass_kernel_spmd

N_CORES = 8
B_TOT = 4096
BS = B_TOT // N_CORES          # 512 batch rows per core
F = 12000
NK = 94                        # k-tiles of 128 features
NP = 47                        # k-pairs (DoubleRow processes 2 k-tiles)
FP = NK * 128                  # 12032
H1, H2 = 4096, 512
NM1 = H1 // 128                # 32
NM2 = H2 // 128                # 4
NK2 = H1 // 128                # 32 mm2 k-tiles
M_HEAD = 5                     # m-tiles computed behind the phase-C frontier
CH = 8                         # w1 chunk size in k-pairs
NCH = (NP + CH - 1) // CH      # 6 chunks (last covers 7 pairs)
XG = 4                         # x-group size in k-pairs
NXG = (NP + XG - 1) // XG      # 12 groups (last has 3)
SELG = 8                       # ssel group size in k-tiles
NSELG = (NK + SELG - 1) // SELG  # 12 groups (last has 6)
LN8 = float(np.log(8.0))

f32 = mybir.dt.float32
bf16 = mybir.dt.bfloat16
f8 = mybir.dt.float8e4
AF = mybir.ActivationFunctionType
DR = mybir.MatmulPerfMode.DoubleRow

E4 = ml_dtypes.float8_e4m3
BF = ml_dtypes.bfloat16

_NC_CACHE = {}


def build_nc(debug=False):
    nc = bacc.Bacc(None, target_bir_lowering=False)

    x8_d = nc.dram_tensor("x8", [NXG, 128, XG, 2, BS], f8, kind="ExternalInput")
    xbf_d = nc.dram_tensor("xbf", [NXG, 128, XG, 2, BS], bf16, kind="ExternalInput")
    wp_d = nc.dram_tensor("wp8", [128, NP, 2, 64], f8, kind="ExternalInput")
    rm_d = nc.dram_tensor("rmab", [50, 2, 128], bf16, kind="ExternalInput")
    ssel_d = nc.dram_tensor("ssel", [NSELG, 128, SELG, 128], bf16, kind="ExternalInput")
    w1_d = nc.dram_tensor("w1q", [NM1, 128, NCH * CH, 2, 2, 128], f8, kind="ExternalInput")
    w2_d = nc.dram_tensor("w2k", [NK2, 128, NM2, 128], bf16, kind="ExternalInput")
    w3_d = nc.dram_tensor("w3h", [128, NM2, 3], bf16, kind="ExternalInput")
    b1_d = nc.dram_tensor("b1g", [128, NM1], f32, kind="ExternalInput")
    b2_d = nc.dram_tensor("b2g", [128, NM2], f32, kind="ExternalInput")
    b3_d = nc.dram_tensor("b3g", [3, 1], f32, kind="ExternalInput")
    out_d = nc.dram_tensor("out", [3, BS], f32, kind="ExternalOutput")
    if debug:
        dbg_y = nc.dram_tensor("dbg_y", [50, BS], bf16, kind="ExternalOutput")
        dbg_lpa = nc.dram_tensor("dbg_lpa", [128, BS], bf16, kind="ExternalOutput")
        dbg_lpb = nc.dram_tensor("dbg_lpb", [128, BS], bf16, kind="ExternalOutput")
        dbg_gt = nc.dram_tensor("dbg_gt", [128, 2, 2, BS], bf16, kind="ExternalOutput")
        dbg_z8 = nc.dram_tensor("dbg_z8", [128, 2, 2, BS], f8, kind="ExternalOutput")
        dbg_zr8 = nc.dram_tensor("dbg_zr8", [128, 2, 2, BS], f8, kind="ExternalOutput")
        dbg_z1 = nc.dram_tensor("dbg_z1", [128, BS], bf16, kind="ExternalOutput")

    with tile.TileContext(nc) as tc, ExitStack() as ctx:
        consts = ctx.enter_context(tc.tile_pool(name="consts", bufs=1))

        b1_sb = consts.tile([128, NM1], f32)
        b2_sb = consts.tile([128, NM2], f32)
        b3_sb = consts.tile([3, 1], f32)
        w3_sb = consts.tile([128, NM2, 3], bf16)
        rm_sb = consts.tile([50, 2, 128], bf16)
        ln8_sb = consts.tile([128, 1], f32)
        ones31 = consts.tile([3, 1], f32)
        ones13 = consts.tile([1, 3], f32)
        nc.any.memset(ln8_sb[:], LN8)
        nc.any.memset(ones31[:], 1.0)
        nc.any.memset(ones13[:], 1.0)

        # persistent pools (lifetime: pre-A through phase E, LIFO-outermost)
        NB = (NP + 1) // 2  # 24 two-pair production blocks (last half-padded)
        zstack = ExitStack()
        z8pool = zstack.enter_context(tc.tile_pool(name="z8p", bufs=NB))
        zr8pool = zstack.enter_context(tc.tile_pool(name="zr8p", bufs=NB))
        z8s, zr8s, z1s = [], [], []

        # phase C..D pools
        dstack = ExitStack()
        psm = dstack.enter_context(tc.tile_pool(name="psm", bufs=1, space="PSUM"))

        # phase C-only pools (close at end of C)
        cstack = ExitStack()
        lpp = cstack.enter_context(tc.tile_pool(name="lpp", bufs=1))
        sselp = cstack.enter_context(tc.tile_pool(name="sselp", bufs=2))
        xbfp = cstack.enter_context(tc.tile_pool(name="xbfp", bufs=2))
        gtp = cstack.enter_context(tc.tile_pool(name="gtp", bufs=2))
        zfp = cstack.enter_context(tc.tile_pool(name="zfp", bufs=3))
        psg = cstack.enter_context(tc.tile_pool(name="psg", bufs=3, space="PSUM"))
        w1hp = cstack.enter_context(tc.tile_pool(name="w1hp", bufs=2))
        psmh = cstack.enter_context(tc.tile_pool(name="psmh", bufs=1, space="PSUM"))

        sseltiles = []
        xbftiles = []

        def xbf_dma(g, qeng):
            if g >= NXG:
                return
            xt = xbfp.tile([128, XG, 2, BS], bf16, tag="xbf")
            qeng.dma_start(xt[:], xbf_d[g])
            xbftiles.append(xt)

        def ssel_dma(g, qeng):
            if g >= NSELG:
                return
            st = sselp.tile([128, SELG, 128], bf16, tag="ssel")
            qeng.dma_start(st[:], ssel_d[g])
            sseltiles.append(st)

        # first ssel group rides the ACT queue (after the odd x8 groups);
        # xbf groups all ride SP right behind the even x8 groups
        pass

        # ---------------- Phase A+B: pools (fp8 DoubleRow) -> Lp tiles
        LpA = lpp.tile([128, BS], bf16, tag="LpA")
        LpB = lpp.tile([128, BS], bf16, tag="LpB")
        # phase A/B PSUM rides the phase-C pools: ypre rotates through the
        # gate-psum tag, Ta/Tb through the head-accumulator tags (same shape)
        with (
            tc.tile_pool(name="x8p", bufs=3) as x8pool,
            tc.tile_pool(name="wpp", bufs=1) as wpp,
        ):
            wpt = wpp.tile([128, NP, 2, 64], f8)
            nc.gpsimd.dma_start(wpt[:], wp_d[:])
            ypre = psg.tile([64, BS], f32, tag="gp", name="ypre")
            for g in range(NXG):
                cnt = min(XG, NP - g * XG)
                x8t = x8pool.tile([128, XG, 2, BS], f8, tag="x8")
                # split the critical x8 stream across SP and ACT queues so
                # issue overhead pipelines with transfers
                (nc.sync if g % 2 == 0 else nc.scalar).dma_start(x8t[:], x8_d[g])
                for i in range(cnt):
                    p = g * XG + i
                    nc.tensor.matmul(
                        ypre[:], wpt[:, p], x8t[:, i],
                        start=(p == 0), stop=(p == NP - 1),
                        perf_mode=DR,
                    )
            ssel_dma(0, nc.scalar)
            nc.scalar.dma_start(rm_sb[:], rm_d[:])
            # y = relu(ypre * 2^-8) in bf16; T = rm^T @ y; Lp = softplus(-T)
            # (y and ea/eb borrow slots from the phase-C rolling pools)
            y_t = zfp.tile([128, 2, BS], bf16, tag="zf", name="y_t")
            y_sb = y_t[0:50, 0]
            nc.scalar.activation(y_sb, ypre[0:50], AF.Relu, scale=float(2.0 ** -8))
            Ta = psmh.tile([128, BS], f32, tag="mm1h_0", bufs=1, name="Ta")
            Tb = psmh.tile([128, BS], f32, tag="mm1h_1", bufs=1, name="Tb")
            nc.tensor.matmul(Ta[:], rm_sb[:, 0], y_sb)
            nc.tensor.matmul(Tb[:], rm_sb[:, 1], y_sb)
            if debug:
                nc.gpsimd.dma_start(dbg_y[:], y_sb)
            e_t = gtp.tile([128, 2, BS], bf16, tag="gt", name="e_t")
            nc.scalar.activation(e_t[:, 0], Ta[:], AF.Exp, scale=-1.0)
            nc.scalar.activation(e_t[:, 1], Tb[:], AF.Exp, scale=-1.0)
            nc.scalar.activation(LpA[:], e_t[:, 0], AF.Ln, bias=1.0)
            nc.scalar.activation(LpB[:], e_t[:, 1], AF.Ln, bias=1.0)

        # ---------------- Phase C: gating + z8/dz8 production + mm1 head
        def w1chunk(m, c0, cnt, pool, tag, qeng, bufs):
            wt = pool.tile([128, CH, 2, 2, 128], f8, tag=tag, bufs=bufs)
            nm = min(cnt, NP - c0)
            qeng.dma_start(wt[:, :nm], w1_d[m, :, c0 : c0 + nm])
            return wt

        def mm1(acc, wt, j, t, zt, start, stop):
            # t: 0 = z8@w8, 1 = z8@dw8, 2 = dz8@w8 (stationary idx 1 only for t=1)
            nc.tensor.matmul(
                acc[:], wt[:, j, 1 if t == 1 else 0], zt[:],
                start=start, stop=stop,
                perf_mode=DR, skip_group_check=True,
            )

        # heads 0..2 use the psmh banks; heads 3..4 borrow the two mm1d
        # banks, which phase D only needs after the head accs are evicted
        accs_head = [
            psmh.tile([128, BS], f32, tag=f"mm1h_{m}", bufs=1, name=f"acch_{m}")
            for m in range(3)
        ] + [
            psm.tile([128, BS], f32, tag="mm1d", bufs=2, name=f"acch_{m}")
            for m in range(3, M_HEAD)
        ]
        # chunk 0 for the head m-tiles rides the (otherwise idle) Pool queue
        wts_head = {
            (m, 0): w1chunk(m, 0, CH, w1hp, f"w1h_{m}", nc.gpsimd, 2)
            for m in range(M_HEAD)
        }
        nc.gpsimd.dma_start(b1_sb[:], b1_d[:])
        nc.gpsimd.dma_start(b2_sb[:], b2_d[:])
        nc.gpsimd.dma_start(b3_sb[:], b3_d[:])
        nc.gpsimd.dma_start(w3_sb[:], w3_d[:])

        def evict_z1(m, acc):
            z1t = z8pool.tile([128, BS], bf16, tag="z1", bufs=6, name=f"z1_{m}")
            nc.scalar.activation(
                z1t[:], acc[:], AF.Relu,
                bias=b1_sb[:, m : m + 1], scale=float(2.0 ** -13),
            )
            z1s.append(z1t)
            if debug and m == 0:
                nc.gpsimd.dma_start(dbg_z1[:], z1t[:])

        def emit_pass1(q):
            for m in range(M_HEAD):
                mm1(accs_head[m], wts_head[(m, q // CH)], q % CH, 0, z8s[q],
                    q == 0, False)

        def emit_passes23(p, last=False):
            c, j = p // CH, p % CH
            for m in range(M_HEAD):
                wt = wts_head[(m, c)]
                mm1(accs_head[m], wt, j, 1, z8s[p], False, False)    # z8 @ dw8
                mm1(accs_head[m], wt, j, 2, zr8s[p], False,
                    last and m == M_HEAD - 1)                        # dz8 @ w8

        if debug:
            nc.gpsimd.dma_start(dbg_lpa[:], LpA[:])
            nc.gpsimd.dma_start(dbg_lpb[:], LpB[:])
        zcs = []
        for b in range(NB):
            # weave streamed inputs into SP with ~2-group lead time
            if b == 0:
                xbf_dma(0, nc.sync)
                xbf_dma(1, nc.sync)
            if b % 2 == 0:
                xbf_dma(b // 2 + 2, nc.sync)
            if b % 4 == 0:
                ssel_dma(b // 4 * 2 + 1, nc.sync)
                ssel_dma(b // 4 * 2 + 2, nc.sync)
                if (b // 4 + 1) * CH < NP:
                    for m in range(M_HEAD):
                        wts_head[(m, b // 4 + 1)] = w1chunk(
                            m, (b // 4 + 1) * CH, CH, w1hp, f"w1h_{m}", nc.sync, 2
                        )

            gt = gtp.tile([128, 2, 2, BS], bf16, tag="gt")
            for i in range(2):
                p = 2 * b + i
                if p >= NP:
                    nc.vector.memzero(gt[:, i])
                    continue
                for s in range(2):
                    k = 2 * p + s
                    gp = psg.tile([128, BS], f32, tag="gp")
                    Lp = LpA if k <= 61 else LpB
                    nc.tensor.matmul(
                        gp[:], sseltiles[k // SELG][:, k % SELG], Lp[:],
                        skip_group_check=True,
                    )
                    nc.scalar.activation(
                        gt[:, i, s], gp[:], AF.Exp, bias=ln8_sb[:, 0:1], scale=-1.0
                    )
            zf = zfp.tile([128, 2, 2, BS], bf16, tag="zf")
            g, h = b // 2, (b % 2) * 2
            nc.vector.tensor_mul(zf[:], xbftiles[g][:, h : h + 2], gt[:])
            z8b = z8pool.tile([128, 2, 2, BS], f8, tag="z8")
            nc.gpsimd.tensor_copy(z8b[:], zf[:])
            zr8b = zr8pool.tile([128, 2, 2, BS], f8, tag="zr8")
            # the sub for block b-1 is emitted AFTER mul(b): DVE is in-order,
            # so this keeps mul(b) from queueing behind a sub that waits on
            # the Pool cast (the cast overlaps the next mul instead)
            zcs.append((zf, z8b, zr8b))
            if b > 0:
                pzf, pz8, pzr = zcs[b - 1]
                nc.vector.tensor_sub(pzr[:], pzf[:], pz8[:])
            if debug and b == 0:
                nc.gpsimd.dma_start(dbg_gt[:], gt[:])
                nc.gpsimd.dma_start(dbg_z8[:], z8b[:])
            if debug and b == 1:
                nc.gpsimd.dma_start(dbg_zr8[:], zcs[0][2][:])
            for i in range(2):
                if 2 * b + i < NP:
                    z8s.append(z8b[:, i])
                    zr8s.append(zr8b[:, i])

            # mm1 head lags the production frontier by 6 pairs (pass 1) and
            # 8 pairs (passes 2/3): the production chain latency is ~2 pairs,
            # so the in-order PE stream never waits on a just-produced tile
            for i in range(2):
                q = 2 * b + i - 6
                if 0 <= q < NP:
                    emit_pass1(q)
                q = 2 * b + i - 8
                if 0 <= q < NP:
                    emit_passes23(q)
        pzf, pz8, pzr = zcs[NB - 1]
        nc.vector.tensor_sub(pzr[:], pzf[:], pz8[:])
        for q in range(NP - 5, NP):
            emit_pass1(q)
        for q in range(NP - 7, NP):
            emit_passes23(q, last=(q == NP - 1))
        for m in range(M_HEAD):
            evict_z1(m, accs_head[m])
        cstack.close()

        # ---------------- Phase D: remaining mm1 m-tiles (full speed).
        # Chunk DMAs are emitted two chunks ahead in the flat (m, c) sequence
        # so the SP queue fires each load exactly when its slot frees.
        estack = ExitStack()
        w1p = estack.enter_context(tc.tile_pool(name="w1p", bufs=3))
        w2p = estack.enter_context(tc.tile_pool(name="w2p", bufs=16))
        psm2 = estack.enter_context(tc.tile_pool(name="psm2", bufs=1, space="PSUM"))
        w2tiles = []
        acc2s = [
            psm2.tile([128, BS], f32, tag=f"mm2_{m2}", name=f"acc2_{m2}")
            for m2 in range(NM2)
        ]

        def emit_mm2(k2):
            for m2 in range(NM2):
                nc.tensor.matmul(
                    acc2s[m2][:], w2tiles[k2][:, m2], z1s[k2][:],
                    start=(k2 == 0), stop=(k2 == NK2 - 1),
                    skip_group_check=True,
                )
        chunks = [(m, c) for m in range(M_HEAD, NM1) for c in range(NCH)]
        wts_d = {}
        wts_d[chunks[0]] = w1chunk(chunks[0][0], chunks[0][1] * CH, CH, w1p, "w1d", nc.sync, 3)
        wts_d[chunks[1]] = w1chunk(chunks[1][0], chunks[1][1] * CH, CH, w1p, "w1d", nc.sync, 3)
        accs_d = {}
        for idx, (m, c) in enumerate(chunks):
            if idx + 2 < len(chunks):
                nm, ncc = chunks[idx + 2]
                wts_d[(nm, ncc)] = w1chunk(nm, ncc * CH, CH, w1p, "w1d", nc.sync, 3)
            if c == 0:
                accs_d[m] = psm.tile([128, BS], f32, tag="mm1d", bufs=2, name=f"accd_{m}")
                # w2 prefetch rides the (idle) Pool queue, 4 tiles per m-sweep
                for k2 in range(len(w2tiles), min(NK2, m + 2)):
                    w2t = w2p.tile([128, NM2, 128], bf16, tag="w2")
                    nc.gpsimd.dma_start(w2t[:], w2_d[k2])
                    w2tiles.append(w2t)
            acc, wt = accs_d[m], wts_d[(m, c)]
            cnt = min(CH, NP - c * CH)
            for j in range(cnt):
                p = c * CH + j
                mm1(acc, wt, j, 0, z8s[p], p == 0, False)
                mm1(acc, wt, j, 2, zr8s[p], False, False)
                mm1(acc, wt, j, 1, z8s[p], False, p == NP - 1)
            if c == 2:
                # interleave mm2 for already-evicted z1 tiles (k2 <= m-1)
                if m == M_HEAD:
                    for k2 in range(M_HEAD):
                        emit_mm2(k2)
                else:
                    emit_mm2(m - 1)
            if c == NCH - 1:
                evict_z1(m, accs_d[m])

        # ---------------- Phase E: mm2 (bf16) + Phase F: mm3 + normalize
        emit_mm2(NK2 - 1)
        with (
            tc.tile_pool(name="z2p", bufs=1) as z2p,
            tc.tile_pool(name="tailp", bufs=1) as tailp,
            tc.tile_pool(name="psf", bufs=1, space="PSUM") as psf,
        ):
            z2_tiles = []
            for m2 in range(NM2):
                z2t = z2p.tile([128, BS], bf16, tag=f"z2_{m2}")
                nc.scalar.activation(
                    z2t[:], acc2s[m2][:], AF.Relu, bias=b2_sb[:, m2 : m2 + 1]
                )
                z2_tiles.append(z2t)

            acc3 = psf.tile([3, BS], f32, tag="f")  # sps/inv3 reuse this bank
            for k3 in range(NM2):
                nc.tensor.matmul(
                    acc3[:], w3_sb[:, k3], z2_tiles[k3][:],
                    start=(k3 == 0), stop=(k3 == NM2 - 1),
                )
            z3 = tailp.tile([3, BS], f32)
            nc.vector.tensor_scalar_add(z3[:], acc3[:], b3_sb[:])
            sq = tailp.tile([3, BS], f32)
            nc.scalar.activation(sq[:], z3[:], AF.Square)
            sps = psf.tile([1, BS], f32, tag="f")
            nc.tensor.matmul(sps[:], ones31[:], sq[:])
            # 1/max(sqrt(s), 1e-12) = min(exp(-0.5*ln(s)), 1e12)
            lns = tailp.tile([1, BS], f32)
            nc.scalar.activation(lns[:], sps[:], AF.Ln)
            inv = tailp.tile([1, BS], f32)
            nc.scalar.activation(inv[:], lns[:], AF.Exp, scale=-0.5)
            nc.vector.tensor_scalar_min(inv[:], inv[:], 1e12)
            inv3 = psf.tile([3, BS], f32, tag="f")
            nc.tensor.matmul(inv3[:], ones13[:], inv[:])
            outt = tailp.tile([3, BS], f32)
            nc.vector.tensor_mul(outt[:], z3[:], inv3[:])
            nc.gpsimd.dma_start(out_d[:], outt[:])

        estack.close()
        dstack.close()
        zstack.close()

    nc.compile()
    return nc


def _prep_shared(conv_w, F_w, w1, b1, w2, b2, w3, b3):
    """Host-side weight layouts shared by all cores."""
    fa = np.arange(F)
    c_i = fa // 4000
    d_i = (fa // 400) % 10
    h_i = (fa // 40) % 10
    w_i = fa % 40

    # pooled conv (x8 fp8, weights scaled 2^8): ypre[j] = sum_f wp[f,j]*x8T[f,b]
    wp = np.zeros((FP, 64), np.float32)
    wp[fa, h_i] += conv_w[c_i] / 400.0 * 256.0
    wp[fa, 10 + w_i] += conv_w[c_i] / 100.0 * 256.0
    wp8 = np.ascontiguousarray(
        wp.reshape(NP, 2, 128, 64).transpose(2, 0, 1, 3)
    ).astype(E4)

    # rm150 channel-grouped: T[c*50+i] = F_w[c]*y[i](sh), T[c*50+10+w'] = F_w[c]*y[10+w']
    rm150 = np.zeros((50, 150), np.float32)
    for c in range(3):
        rm150[np.arange(10), c * 50 + np.arange(10)] = F_w[c]
        rm150[10 + np.arange(40), c * 50 + 10 + np.arange(40)] = F_w[c]
    rmab = np.stack([rm150[:, 0:128], rm150[:, 22:150]], axis=1)  # [50, 2, 128]
    rmab = np.ascontiguousarray(rmab).astype(BF)

    # per-k selection over the channel-grouped Lp window (A: rows 0..127 for
    # k<=61, B: rows 22..149 for k>=62); += handles the d==h coincidence
    sel = np.zeros((NK, 128, 128), np.float32)
    rows = np.stack([c_i * 50 + d_i, c_i * 50 + h_i, c_i * 50 + 10 + w_i])
    for k in range(NK):
        base = 0 if k <= 61 else 22
        jj = np.arange(k * 128, min((k + 1) * 128, F))
        for r3 in rows:
            np.add.at(sel[k], (r3[jj] - base, jj - k * 128), 1.0)
    selg = np.zeros((NSELG * SELG, 128, 128), np.float32)
    selg[:NK] = sel
    ssel = np.ascontiguousarray(
        selg.reshape(NSELG, SELG, 128, 128).transpose(0, 2, 1, 3)
    ).astype(BF)

    # w1: fp8 main + fp8 residual at shared scale 2^10, k-pair interleaved
    w1s = (w1.astype(np.float64) * 1024.0).astype(np.float32)
    w1p_ = np.zeros((H1, NCH * CH * 256), np.float32)
    w1p_[:, :F] = w1s[:, :F]
    w8 = w1p_.astype(E4)
    wr8 = (w1p_ - w8.astype(np.float32)).astype(E4)

    # [H1, FPc] -> [m, mcol, pair, s, p] -> [m, p, pair, s, mcol]
    def lay(a):
        return a.reshape(NM1, 128, NCH * CH, 2, 128).transpose(0, 4, 2, 3, 1)

    w1q = np.ascontiguousarray(np.stack([lay(w8), lay(wr8)], axis=3))
    # shape [NM1, 128, NCH*CH(pair), 2(t), 2(s), 128]

    w2k = np.ascontiguousarray(
        w2.reshape(NM2, 128, NK2, 128).transpose(2, 3, 0, 1)
    ).astype(BF)
    w3h = np.ascontiguousarray(w3.reshape(3, NM2, 128).transpose(2, 1, 0)).astype(BF)

    return {
        "wp8": wp8,
        "rmab": rmab,
        "ssel": ssel,
        "w1q": w1q,
        "w2k": w2k,
        "w3h": w3h,
        "b1g": np.ascontiguousarray(b1.reshape(NM1, 128).T),
        "b2g": np.ascontiguousarray(b2.reshape(NM2, 128).T),
        "b3g": np.ascontiguousarray(b3.reshape(3, 1)),
    }


def make_in_maps(x, conv_w, F_w, w1, b1, w2, b2, w3, b3):
    x = np.asarray(x, np.float32).reshape(B_TOT, F)
    shared = _prep_shared(
        np.asarray(conv_w, np.float32).reshape(3),
        np.asarray(F_w, np.float32).reshape(3),
        np.asarray(w1, np.float32),
        np.asarray(b1, np.float32),
        np.asarray(w2, np.float32),
        np.asarray(b2, np.float32),
        np.asarray(w3, np.float32),
        np.asarray(b3, np.float32),
    )
    in_maps = []
    for c in range(N_CORES):
        xs = x[c * BS : (c + 1) * BS]
        xt = np.zeros((NXG * XG * 2 * 128, BS), np.float32)
        xt[:F] = xs.T
        # [f, b] -> [g, k(XG), s, p, b] -> [g, p, k, s, b]
        xt = xt.reshape(NXG, XG, 2, 128, BS).transpose(0, 3, 1, 2, 4)
        m = dict(shared)
        m["x8"] = np.ascontiguousarray(xt).astype(E4)
        m["xbf"] = np.ascontiguousarray(xt).astype(BF)
        in_maps.append(m)
    return in_maps


def get_nc():
    if "nc" not in _NC_CACHE:
        _NC_CACHE["nc"] = build_nc()
    return _NC_CACHE["nc"]


def kernel(**inputs) -> np.ndarray:
    nc = get_nc()
    in_maps = make_in_maps(**inputs)
    res = run_bass_kernel_spmd(nc, in_maps, core_ids=list(range(N_CORES)))
    out = np.concatenate([r["out"] for r in res.results], axis=1)  # [3, 4096]
    return np.ascontiguousarray(out.T, dtype=np.float32)
